# revision 33
# baseline (speedup 1.0000x reference)
"""DeepseekV2 decoder layer — Trainium2 Bass kernel (data-parallel over tokens).

Strategy:
- 8 cores, token-sharded (512 tokens each), identical program, no collectives.
- Activations in transposed layout [feature, token]; weights host-pretransposed
  to K-major tiles; all matmul operands bf16 (full-rate, half the HBM traffic
  of fp32), PSUM accumulation in fp32.
- attention collapses to one matmul: hidden = x + s1 * ((Wo@Wq)' @ x), with
  the RMSNorm scale s1 applied AFTER the matmul (per-token column scale
  commutes with the contraction). k-outer loop over two 8-o-tile groups keeps
  all 8 PSUM banks accumulating, so the first matmul needs only the first
  128-row slice of x — the PE starts ~2us in instead of waiting for all of x.
- two HWDGE queues: sync streams x + gate/up/down weights; the scalar engine's
  ring streams attention weight granules + output stores, so neither starves
  the other. RMSNorm variance reduction runs on the idle gpsimd engine
  (partition_all_reduce) — no PE matmuls, no PSUM bank.
- group drain: PSUM banks are freed by a tight run of DVE muls into the acc
  tile (idle during attention), then hid/var2 chores follow — the next
  o-group's matmuls restart after ~1 DVE op instead of the full drain.
- in_w folded into W_qo columns, post_w folded into Wg/Wu columns.
- MLP intermediate dim padded 10944->11008 (86 tiles), two 43-tile phases:
  gate/up matmuls into PSUM, DVE applies s2 + silu + mul into bf16 gu tiles;
  down accumulated across the two phases in SBUF fp32; final residual folded
  into the last down pass, streamed straight to DRAM per o-tile; the very
  last o-tile is processed in two column halves so its output DMA overlaps
  the remaining matmuls.
"""

import sys
import numpy as np

sys.path.insert(0, "/opt/trn_rl_repo")
sys.path.insert(0, "/root/.axon_site/_ro/trn_rl_repo")

import concourse.bass as bass
import concourse.bass_isa as bass_isa
import concourse.mybir as mybir
import concourse.tile as tile
from concourse import bacc

P = 128
T_C = 512          # tokens per core
H = 2048
HO = H // P        # 16
I_RAW = 10944
ION = 86           # ceil(10944/128) -> padded tiles
I_PAD = ION * P    # 11008
EPS = 1e-6
N_CORES = 8
T_FULL = 4096
PHASES = (43, 43)  # i-tile counts per MLP phase
OG = 8             # o-tiles per attention PSUM group
# (k-tiles, queue) per attention weight granule. k0 rides the sync ring (the
# scalar ring is blocked by the ACT table load at kernel start); k1-7 stream
# on the scalar ring, k8-15 on sync. All small so every slot dispatches early.
GSZ = ((1, "gp"), (1, "gp"), (2, "sc"), (2, "sc"), (2, "sc"),
       (2, "sy"), (2, "sy"), (2, "sy"), (2, "sy"))

f32 = mybir.dt.float32
bf16 = mybir.dt.bfloat16


def build_program(n_cores=N_CORES):
    nc = bacc.Bacc("TRN2", target_bir_lowering=False, debug=False,
                   num_devices=n_cores)
    xt_d = nc.dram_tensor("xt", [P, HO * T_C], bf16, kind="ExternalInput").ap()
    # per-group k-major attn weights, partition-contiguous so weight granules
    # have multi-KB DMA descriptor runs:
    #   wqoa[p, k*1024 + a*128 + b] = W_qo'[a*128+b, k*128+p]   (o-tiles 0..7)
    #   wqob likewise for o-tiles 8..15
    wqoa_d = nc.dram_tensor("wqoa", [P, HO * OG * P], bf16,
                            kind="ExternalInput").ap()
    wqob_d = nc.dram_tensor("wqob", [P, HO * OG * P], bf16,
                            kind="ExternalInput").ap()
    wgu_d = nc.dram_tensor("wgu", [ION, P, 2 * HO * P], bf16,
                           kind="ExternalInput").ap()
    wd_d = nc.dram_tensor("wd", [HO, P, ION * P], bf16, kind="ExternalInput").ap()
    out_d = nc.dram_tensor("out", [HO, P, T_C], f32, kind="ExternalOutput").ap()

    with tile.TileContext(nc) as tc:
        with (
            tc.tile_pool(name="hidp", bufs=1) as hidp,      # hid bf16 16KB
            tc.tile_pool(name="accp", bufs=1) as accp,      # down acc f32 32KB
            tc.tile_pool(name="gup", bufs=1) as gup,        # gu phase / x staging
            tc.tile_pool(name="watt", bufs=6) as watt,      # attn weight granules
            tc.tile_pool(name="wts", bufs=4) as wts,        # gate/up weights 8KB
            tc.tile_pool(name="wdp", bufs=3) as wdp,        # down weights 10.75KB
            tc.tile_pool(name="scr", bufs=6) as scr,        # [P,512] scratch
            tc.tile_pool(name="vsc", bufs=6) as vsc,        # var/scale tiles
            tc.tile_pool(name="cst", bufs=1) as cst,
            tc.tile_pool(name="mps", bufs=8, space="PSUM") as mps,   # all 8 banks
        ):
            eps_t = cst.tile([P, 1], f32, name="eps_t")
            nc.vector.memset(eps_t[:], EPS)

            def rms_scale(sq_acc, name):
                """sq_acc [P,T_C] partial squares -> s_b [P,T_C] = rsqrt(mean+eps)."""
                red = vsc.tile([P, T_C], f32, name=f"red_{name}", tag="v")
                nc.gpsimd.partition_all_reduce(red[:], sq_acc[:], channels=P,
                                               reduce_op=bass_isa.ReduceOp.add)
                rt = vsc.tile([P, T_C], f32, name=f"rt_{name}", tag="v")
                nc.scalar.activation(rt[:], red[:],
                                     mybir.ActivationFunctionType.Sqrt,
                                     bias=eps_t[:], scale=1.0 / H)
                s_b = vsc.tile([P, T_C], f32, name=f"s_{name}", tag="v")
                rsc = scr.tile([P, T_C], f32, name=f"rsc_{name}", tag="scr")
                nc.vector.reciprocal_approx_accurate(s_b[:], rt[:], rsc[:])
                return s_b

            # ---- stage x once via sync queue (borrows the gu slot) ----
            xfull = gup.tile([P, HO, T_C], bf16, name="xfull", tag="gu")
            xt_3d = xt_d.rearrange("p (ho t) -> p ho t", ho=HO)
            done_q = 0
            for qsz in (2, 2, 4, 8):
                nc.sync.dma_start(
                    out=xfull[:, done_q:done_q + qsz, :],
                    in_=xt_3d[:, done_q:done_q + qsz, :])
                done_q += qsz

            # ---- var1 = sum(x^2): all-DVE, two accumulators to halve the
            # serial chain (the scalar queue stays free for weight DMAs) ----
            sq1e = vsc.tile([P, T_C], f32, name="sq1e", tag="v")
            sq1o = vsc.tile([P, T_C], f32, name="sq1o", tag="v")
            nc.vector.tensor_mul(sq1e[:], xfull[:, 0, :], xfull[:, 0, :])
            nc.vector.tensor_mul(sq1o[:], xfull[:, 1, :], xfull[:, 1, :])
            for k in range(2, HO):
                sq = scr.tile([P, T_C], bf16, name="sqs", tag="scr")
                nc.vector.tensor_mul(sq[:], xfull[:, k, :], xfull[:, k, :])
                dst = sq1e if k % 2 == 0 else sq1o
                nc.vector.tensor_add(dst[:], dst[:], sq[:])
            nc.vector.tensor_add(sq1e[:], sq1e[:], sq1o[:])
            sq1 = sq1e

            # ---- attn: hid = x + s1 * (W_qo' @ x), k-outer in two o-groups ----
            # acc doubles as the post-matmul scale buffer during attention.
            hid = hidp.tile([P, HO, T_C], bf16, name="hid", tag="hid")
            acc = accp.tile([P, HO, T_C], f32, name="acc", tag="acc")
            sq2e = vsc.tile([P, T_C], f32, name="sq2e", tag="v")
            sq2o = vsc.tile([P, T_C], f32, name="sq2o", tag="v")
            s1_b = None
            for g in range(HO // OG):
                wsrc = wqoa_d if g == 0 else wqob_d
                ps_g = [mps.tile([P, T_C], f32, name=f"att_ps{g}_{ol}", tag="mm")
                        for ol in range(OG)]
                k0 = 0
                for gi, (sz, qn) in enumerate(GSZ):
                    eng = {"sc": nc.scalar, "sy": nc.sync,
                           "gp": nc.gpsimd}[qn]
                    if g == 0 and gi == 0:
                        # first granule split in two 128KB halves so the
                        # earliest matmuls start as soon as the first lands
                        hw_ = OG // 2
                        wts_2 = []
                        for hh in range(2):
                            wt = watt.tile([P, hw_, P], bf16,
                                           name=f"wqo_t0{hh}", tag="wa")
                            eng.dma_start(
                                out=wt[:],
                                in_=wsrc[:, hh * hw_ * P:(hh + 1) * hw_ * P])
                            wts_2.append(wt)
                        for ol in range(OG):
                            nc.tensor.matmul(ps_g[ol][:],
                                             lhsT=wts_2[ol // hw_][:, ol % hw_, :],
                                             rhs=xfull[:, 0, :],
                                             start=True, stop=False)
                        k0 += sz
                        continue
                    wt = watt.tile([P, sz * OG, P], bf16, name="wqo_t", tag="wa")
                    eng.dma_start(
                        out=wt[:],
                        in_=wsrc[:, k0 * OG * P:(k0 + sz) * OG * P])
                    for kl in range(sz):
                        k = k0 + kl
                        for ol in range(OG):
                            nc.tensor.matmul(ps_g[ol][:],
                                             lhsT=wt[:, kl * OG + ol, :],
                                             rhs=xfull[:, k, :],
                                             start=(k == 0), stop=(k == HO - 1))
                    k0 += sz
                if g == 0:
                    # var1 scale chain: gpsimd AR + ACT sqrt queued after the
                    # group-A granule DMAs on the scalar ring, ready ~15us
                    s1_b = rms_scale(sq1, "1")
                # free the PSUM banks with a tight run of muls into acc ...
                for ol in range(OG):
                    o = g * OG + ol
                    nc.vector.tensor_mul(acc[:, o, :], ps_g[ol][:], s1_b[:])
                # ... then the hid/var2 chores
                for ol in range(OG):
                    o = g * OG + ol
                    nc.vector.tensor_add(hid[:, o, :], acc[:, o, :],
                                         xfull[:, o, :])
                for ol in range(OG):
                    o = g * OG + ol
                    if o < 2:
                        dst = sq2e if o == 0 else sq2o
                        nc.vector.tensor_mul(dst[:], hid[:, o, :],
                                             hid[:, o, :])
                    else:
                        sq = scr.tile([P, T_C], bf16, name="sqs2", tag="scr")
                        nc.vector.tensor_mul(sq[:], hid[:, o, :], hid[:, o, :])
                        dst = sq2e if o % 2 == 0 else sq2o
                        nc.vector.tensor_add(dst[:], dst[:], sq[:])
            nc.vector.tensor_add(sq2e[:], sq2e[:], sq2o[:])
            s2_b = rms_scale(sq2e, "2")

            # ---- MLP in two i-phases (s2 folded into gate/up outputs) ----
            i0 = 0
            for ph, NH in enumerate(PHASES):
                last_ph = ph == len(PHASES) - 1
                gu = gup.tile([P, NH, T_C], bf16, name="gu", tag="gu")
                for il in range(NH):
                    i = i0 + il
                    wgu_t = wts.tile([P, 2 * HO, P], bf16, name="wgu_t", tag="w")
                    nc.sync.dma_start(
                        out=wgu_t[:],
                        in_=wgu_d[i].rearrange("p (a b) -> p a b", a=2 * HO))
                    psg = mps.tile([P, T_C], f32, name="g_ps", tag="mm")
                    for k in range(HO):
                        nc.tensor.matmul(psg[:], lhsT=wgu_t[:, k, :],
                                         rhs=hid[:, k, :],
                                         start=(k == 0), stop=(k == HO - 1))
                    psu = mps.tile([P, T_C], f32, name="u_ps", tag="mm")
                    for k in range(HO):
                        nc.tensor.matmul(psu[:], lhsT=wgu_t[:, HO + k, :],
                                         rhs=hid[:, k, :],
                                         start=(k == 0), stop=(k == HO - 1))
                    g2 = scr.tile([P, T_C], f32, name="g2", tag="scr")
                    nc.vector.tensor_mul(g2[:], psg[:], s2_b[:])
                    gsig = scr.tile([P, T_C], f32, name="gsig", tag="scr")
                    nc.scalar.activation(gsig[:], g2[:],
                                         mybir.ActivationFunctionType.Sigmoid)
                    gact = scr.tile([P, T_C], f32, name="gact", tag="scr")
                    nc.vector.tensor_mul(gact[:], g2[:], gsig[:])
                    u2 = scr.tile([P, T_C], f32, name="u2", tag="scr")
                    nc.vector.tensor_mul(u2[:], psu[:], s2_b[:])
                    nc.vector.tensor_mul(gu[:, il, :], gact[:], u2[:])

                # down for this phase: acc[o] (+)= Wd[:, phase] @ gu
                for o in range(HO):
                    wd_t = wdp.tile([P, NH, P], bf16, name="wd_t", tag="wd")
                    nc.sync.dma_start(
                        out=wd_t[:],
                        in_=wd_d[o, :, i0 * P:(i0 + NH) * P].rearrange(
                            "p (a b) -> p a b", a=NH))
                    final_o = last_ph and o == HO - 1
                    if not final_o:
                        ps = mps.tile([P, T_C], f32, name="d_ps", tag="mm")
                        for kk in range(NH):
                            nc.tensor.matmul(ps[:], lhsT=wd_t[:, kk, :],
                                             rhs=gu[:, kk, :],
                                             start=(kk == 0), stop=(kk == NH - 1))
                        if not last_ph:
                            nc.vector.tensor_copy(acc[:, o, :], ps[:])
                        else:
                            fin = scr.tile([P, T_C], f32, name="fin", tag="scr")
                            nc.vector.tensor_add(fin[:], ps[:], acc[:, o, :])
                            fin2 = scr.tile([P, T_C], f32, name="fin2", tag="scr")
                            nc.vector.tensor_add(fin2[:], fin[:], hid[:, o, :])
                            nc.scalar.dma_start(out=out_d[o], in_=fin2[:])
                    else:
                        # last o-tile: two column halves so the first half's
                        # output DMA overlaps the second half's matmuls
                        ps = mps.tile([P, T_C], f32, name="d_ps", tag="mm")
                        NQ = 2
                        TH = T_C // NQ
                        for h in range(NQ):
                            sl = slice(h * TH, (h + 1) * TH)
                            for kk in range(NH):
                                nc.tensor.matmul(ps[:, sl], lhsT=wd_t[:, kk, :],
                                                 rhs=gu[:, kk, sl],
                                                 start=(kk == 0),
                                                 stop=(kk == NH - 1))
                            fin = scr.tile([P, TH], f32, name="finh", tag="scr")
                            nc.vector.tensor_add(fin[:], ps[:, sl],
                                                 acc[:, o, sl])
                            fin2 = scr.tile([P, TH], f32, name="finh2",
                                            tag="scr")
                            nc.vector.tensor_add(fin2[:], fin[:], hid[:, o, sl])
                            nc.scalar.dma_start(out=out_d[o, :, sl], in_=fin2[:])
                i0 += NH

    nc.compile()
    return nc


# ---------------- host-side data prep ----------------

def _to_bf16(a):
    import ml_dtypes
    return np.asarray(a, dtype=np.float32).astype(ml_dtypes.bfloat16)


def tile_w(A):
    """A [O, Hin] -> [on, P(hin_i), ho*P] K-major tiles."""
    O, Hin = A.shape
    on, ho = O // P, Hin // P
    return np.ascontiguousarray(
        A.T.reshape(ho, P, on, P).transpose(2, 1, 0, 3)
    ).reshape(on, P, ho * P)


def ktile_group(A):
    """A [OG*P, Hin] -> [P, hin_tiles*OG*P] partition-contiguous k-major:
    out[p, k*OG*P + a*P + b] = A[a*P+b, k*P+p]."""
    O, Hin = A.shape
    assert O == OG * P
    return np.ascontiguousarray(
        A.reshape(OG, P, Hin // P, P).transpose(3, 2, 0, 1)
    ).reshape(P, (Hin // P) * OG * P)


def prep_inputs(x, in_w, post_w, Wq, Wo, Wg, Wu, Wd):
    """Returns (shared weight map, per-core x maps list)."""
    W_qo = (Wo.astype(np.float64) @ Wq.astype(np.float64))
    W_qo = (W_qo * in_w.astype(np.float64)[None, :]).astype(np.float32)
    Wg_f = (Wg.astype(np.float64) * post_w.astype(np.float64)[None, :]).astype(np.float32)
    Wu_f = (Wu.astype(np.float64) * post_w.astype(np.float64)[None, :]).astype(np.float32)
    pad = np.zeros((I_PAD - I_RAW, H), np.float32)
    wg_t = tile_w(np.concatenate([Wg_f, pad], 0))      # [ION, P, HO*P]
    wu_t = tile_w(np.concatenate([Wu_f, pad], 0))
    wgu = np.concatenate([wg_t, wu_t], axis=2)         # [ION, P, 2*HO*P]
    Wd_p = np.concatenate([Wd.astype(np.float32),
                           np.zeros((H, I_PAD - I_RAW), np.float32)], 1)
    wmap = {
        "wqoa": _to_bf16(ktile_group(W_qo[:OG * P])),
        "wqob": _to_bf16(ktile_group(W_qo[OG * P:])),
        "wgu": _to_bf16(np.ascontiguousarray(wgu)),
        "wd": _to_bf16(tile_w(Wd_p)),
    }
    xf = np.ascontiguousarray(x.reshape(T_FULL, H).astype(np.float32).T)  # [H, T]
    xmaps = []
    for c in range(N_CORES):
        xc = xf[:, c * T_C:(c + 1) * T_C]                      # [H, T_C]
        xc = np.ascontiguousarray(
            xc.reshape(HO, P, T_C).transpose(1, 0, 2)).reshape(P, HO * T_C)
        xmaps.append({"xt": _to_bf16(xc)})
    return wmap, xmaps


def assemble_output(core_outs):
    """core_outs: list of 8 arrays [HO, P, T_C] -> [2, 2048, 2048] fp32."""
    cols = [o.reshape(H, T_C) for o in core_outs]
    outT = np.concatenate(cols, axis=1)          # [H, T_FULL]
    return np.ascontiguousarray(outT.T).reshape(2, T_FULL // 2, H).astype(np.float32)


# ---------------- public entry point ----------------

_NC_CACHE = {}


def _get_program():
    if "nc" not in _NC_CACHE:
        _NC_CACHE["nc"] = build_program()
    return _NC_CACHE["nc"]


def kernel(x, positions, in_w, post_w, Wq, Wo, Wg, Wu, Wd):
    """Full DeepseekV2 decoder layer on 8 NeuronCores. positions is unused by
    the reference computation (no rotary), accepted for signature parity."""
    nc = _get_program()
    wmap, xmaps = prep_inputs(
        np.asarray(x), np.asarray(in_w), np.asarray(post_w), np.asarray(Wq),
        np.asarray(Wo), np.asarray(Wg), np.asarray(Wu), np.asarray(Wd))
    in_maps = [{**wmap, **xm} for xm in xmaps]
    from concourse.bass_utils import run_bass_kernel_spmd
    res = run_bass_kernel_spmd(nc, in_maps, core_ids=list(range(N_CORES)),
                               trace=False)
    outs = [np.asarray(r["out"], dtype=np.float32) for r in res.results]
    return assemble_output(outs)
